# revision 55
# baseline (speedup 1.0000x reference)
"""BotNet-style multi-head 2D attention with relative position logits, on 8 trn2 cores.

Distribution: data-parallel over batch (B=16 -> 2 per core); all 4 heads +
the rel-pos skew handled on-core.

Per (batch, head) pair the kernel computes, fully on-chip:
    logits = (q*SCALE) @ k^T + skew_w(q @ relw^T) + skew_h(q @ relh^T)
    W      = exp(logits);  W /= rowsum(W)   (softmax without max-subtract:
             logits are O(10) here, exp() is safe in fp32)
    out^T  = V^T @ W^T     (accumulated over key chunks in PSUM)

The rel-pos skew (per-query-row shift) is done with a DRAM round-trip whose
read-back access pattern bakes in the shift, then the per-row [64,128] skewed
tile is added into the logits PSUM via a matmul against a constant 0/1
selector matrix (contraction over the 32 width / 32 height rel positions).
"""

import numpy as np
import ml_dtypes

import concourse.bass as bass
import concourse.mybir as mybir
import concourse.tile as tile
from concourse import bacc
from concourse.ap import AP
from concourse.bass_utils import run_bass_kernel_spmd

FP32 = mybir.dt.float32
BF16 = mybir.dt.bfloat16
AF = mybir.ActivationFunctionType

import os
ABLATE = set(os.environ.get("KERNEL_ABLATE", "").split(","))

NCORES = 8
B_PER_CORE = 2
HEADS = 4
D = 128          # qk and v head dim
C = 512          # input channels
H = W = 32
L = H * W        # 1024 tokens
RC = L // 128    # 8 row chunks of 128 tokens
CC = C // 128    # 4 contraction chunks for the projections
SCALE = D ** (-0.5)
NREL = 2 * W - 1  # 63


def _sel_matrix():
    # sel[k, i*32+j]: k<32 -> (j == k); k>=32 -> (i == k-32)
    sel = np.zeros((64, L), np.float32)
    ii, jj = np.divmod(np.arange(L), W)
    for k in range(32):
        sel[k, jj == k] = 1.0
        sel[32 + k, ii == k] = 1.0
    return sel.astype(ml_dtypes.bfloat16)


def build_bass(iters=1):
    nc = bacc.Bacc()
    fmap = nc.declare_dram_parameter("fmap", [B_PER_CORE, C, L], FP32, isOutput=False)
    wqk = nc.declare_dram_parameter("w_qk", [2 * HEADS * D, C], FP32, isOutput=False)
    wv = nc.declare_dram_parameter("w_v", [HEADS * D, C], FP32, isOutput=False)
    relh = nc.declare_dram_parameter("rel_height", [NREL, D], FP32, isOutput=False)
    relw = nc.declare_dram_parameter("rel_width", [NREL, D], FP32, isOutput=False)
    out = nc.declare_dram_parameter("out", [B_PER_CORE, HEADS * D, L], FP32, isOutput=True)

    sel_const = nc.inline_tensor(_sel_matrix(), name="sel_const")
    ident_const = nc.inline_tensor(np.eye(128, dtype=ml_dtypes.bfloat16), name="ident_const")

    with tile.TileContext(nc) as tc:
        if iters == 1:
            _body(tc, fmap, wqk, wv, relh, relw, out, sel_const, ident_const)
        else:
            with tc.For_i(0, iters, 1):
                _body(tc, fmap, wqk, wv, relh, relw, out, sel_const, ident_const)
    nc.finalize()
    return nc


def _body(tc, fmap, wqk, wv, relh, relw, out, sel_const, ident_const):
    nc = tc.nc
    import contextlib

    ctx = contextlib.ExitStack()
    with ctx:
        persist = ctx.enter_context(tc.tile_pool(name="persist", bufs=1))
        batch_p = ctx.enter_context(tc.tile_pool(name="batch", bufs=2))
        pair_p = ctx.enter_context(tc.tile_pool(name="pair", bufs=2))
        rel_p = ctx.enter_context(tc.tile_pool(name="rel", bufs=3))
        out_p = ctx.enter_context(tc.tile_pool(name="out", bufs=2))
        wt_p = ctx.enter_context(tc.tile_pool(name="wtsb", bufs=2))
        small = ctx.enter_context(tc.tile_pool(name="small", bufs=2))
        dram_p = ctx.enter_context(tc.tile_pool(name="dram", bufs=3, space="DRAM"))

        ps_big = ctx.enter_context(tc.tile_pool(name="ps_big", bufs=2, space="PSUM"))
        ps_wt = ctx.enter_context(tc.tile_pool(name="ps_wt", bufs=2, space="PSUM"))
        ps_av = ctx.enter_context(tc.tile_pool(name="ps_av", bufs=1, space="PSUM"))

        # ---- constants to SBUF ----
        ident = persist.tile([128, 128], BF16, tag="ident")
        nc.sync.dma_start(out=ident, in_=ident_const[:])
        sel = persist.tile([64, L], BF16, tag="sel")
        nc.sync.dma_start(out=sel, in_=sel_const[:])

        # ---- weight prep: transpose + cast to bf16 (scale folded into q) ----
        # wqk rows: [0,512) = q (scaled), [512,1024) = k
        # single gpsimd DMA per weight (casts fp32->bf16 in flight):
        # [128, oc*512+c] <- w[oc*128+p, c]
        hwload = "hwload" in ABLATE
        wq_all = persist.tile([128, 8 * C], BF16, tag="wqldb")
        wv_all = persist.tile([128, 4 * C], BF16, tag="wvldb")
        if hwload:
            # HWDGE fp32 loads + engine casts: keeps the (slow) SWDGE
            # descriptor generation off the kernel-start critical path
            stage_p = ctx.enter_context(tc.tile_pool(name="stage", bufs=1))
            wq32 = stage_p.tile([128, 8 * C], FP32, tag="wstage")
            nc.sync.dma_start(
                out=wq32.rearrange("p (a c) -> p a c", a=8),
                in_=wqk[:].rearrange("(a p) c -> p a c", p=128))
            nc.vector.tensor_copy(wq_all, wq32)
            wv32 = stage_p.tile([128, 8 * C], FP32, tag="wstage")
            nc.sync.dma_start(
                out=wv32[:, 0:4 * C].rearrange("p (a c) -> p a c", a=4),
                in_=wv[:].rearrange("(a p) c -> p a c", p=128))
            nc.vector.tensor_copy(wv_all, wv32[:, 0:4 * C])
        else:
            nc.gpsimd.dma_start(
                out=wq_all.rearrange("p (a c) -> p a c", a=8),
                in_=wqk[:].rearrange("(a p) c -> p a c", p=128))
        wq_bf = [wq_all[:, oc * C:(oc + 1) * C] for oc in range(8)]
        wv_bf = [wv_all[:, oc * C:(oc + 1) * C] for oc in range(4)]

        # fmap loads issued on the Pool (SWDGE) queue right after wq so the
        # batch-0 chunks land while PE transposes weights; wv (not needed
        # until the v projections ~30us in) queues behind them
        fm_tiles = {}

        def load_fmap(b):
            fm_bf = []
            for cc in range(CC):
                fbf = batch_p.tile([128, L], BF16, tag=f"fmbf_{cc}")
                if hwload:
                    f32 = batch_p.tile([128, L], FP32, tag="fm32")
                    nc.sync.dma_start(out=f32,
                                      in_=fmap[b, cc * 128:(cc + 1) * 128, :])
                    nc.vector.tensor_copy(fbf, f32)
                else:
                    nc.gpsimd.dma_start(out=fbf,
                                        in_=fmap[b, cc * 128:(cc + 1) * 128, :])
                fm_bf.append(fbf)
            fm_tiles[b] = fm_bf

        load_fmap(0)
        if not hwload:
            nc.gpsimd.dma_start(
                out=wv_all.rearrange("p (a c) -> p a c", a=4),
                in_=wv[:].rearrange("(a p) c -> p a c", p=128))

        wqkT = []   # per cc: [128(c), 1024(o)] bf16, q-half pre-scaled
        for cc in range(CC):
            ps = ps_wt.tile([128, 1024], BF16, tag="ps_wt")
            for oc in range(8):
                nc.tensor.transpose(
                    ps[:, oc * 128:(oc + 1) * 128],
                    wq_bf[oc][:, cc * 128:(cc + 1) * 128],
                    ident,
                )
            t = persist.tile([128, 1024], BF16, tag=f"wqkT{cc}")
            nc.vector.tensor_scalar_mul(t[:, 0:512], ps[:, 0:512], float(SCALE))
            nc.vector.tensor_copy(t[:, 512:1024], ps[:, 512:1024])
            wqkT.append(t)

        wvT = []    # per cc: [128(c), 512(o)] bf16
        for cc in range(CC):
            ps = ps_wt.tile([128, 1024], BF16, tag="ps_wt")
            for oc in range(4):
                nc.tensor.transpose(
                    ps[:, oc * 128:(oc + 1) * 128],
                    wv_bf[oc][:, cc * 128:(cc + 1) * 128],
                    ident,
                )
            t = persist.tile([128, 512], BF16, tag=f"wvT{cc}")
            nc.vector.tensor_copy(t, ps[:, 0:512])
            wvT.append(t)

        # rel tables transposed: [128(d), 63] bf16
        relT_tabs = []
        for name, src in (("relw", relw), ("relh", relh)):
            tbf = small.tile([NREL, D], BF16, tag=f"{name}b")
            nc.gpsimd.dma_start(out=tbf, in_=src[:])
            ps = ps_wt.tile([128, 1024], BF16, tag="ps_wt")
            nc.tensor.transpose(ps[:, 0:NREL], tbf, ident[0:NREL, 0:NREL])
            t = persist.tile([128, NREL], BF16, tag=f"{name}T")
            nc.scalar.activation(t, ps[:, 0:NREL], AF.Copy)
            relT_tabs.append(t)
        relwT, relhT = relT_tabs

        # ---- projections (emitted per batch; pair-0's rel round-trip is
        # issued between the two batches so its DRAM latency hides under
        # batch-1's projection matmuls) ----
        qT = {}   # (b, h) -> [128(d), 1024(l)] bf16  (pre-scaled by SCALE)
        kT = {}
        vT = {}   # (b, lc) -> [128(l), 512(h*d)] bf16

        def project(b):
            fm_bf = fm_tiles[b]
            # q/k: out[o_chunk, l] ; o = (q: h*128+d | k: 512 + h*128+d)
            for oc in range(8):
                ps = ps_big.tile([128, L], FP32, tag="big")
                for s in (slice(0, 512), slice(512, 1024)):
                    for cc in range(CC):
                        nc.tensor.matmul(
                            ps[:, s],
                            wqkT[cc][:, oc * 128:(oc + 1) * 128],
                            fm_bf[cc][:, s],
                            start=(cc == 0),
                            stop=(cc == CC - 1),
                        )
                dst = batch_p.tile([128, L], BF16,
                                   tag=f"{'q' if oc < 4 else 'k'}T{oc % 4}")
                if oc < 4:
                    nc.vector.tensor_copy(dst, ps)
                    qT[(b, oc)] = dst
                else:
                    nc.scalar.activation(dst, ps, AF.Copy)
                    kT[(b, oc - 4)] = dst
            # v^T: out[l_chunk, h*d]
            for lc in range(RC):
                ps = ps_big.tile([128, L], FP32, tag="big")
                for cc in range(CC):
                    nc.tensor.matmul(
                        ps[:, 0:512],
                        fm_bf[cc][:, lc * 128:(lc + 1) * 128],
                        wvT[cc],
                        start=(cc == 0),
                        stop=(cc == CC - 1),
                    )
                dst = batch_p.tile([128, 512], BF16, tag=f"vT{lc}")
                nc.vector.tensor_copy(dst, ps[:, 0:512])
                vT[(b, lc)] = dst

        # ---- attention pairs ----
        pairs = [(b, h) for b in range(B_PER_CORE) for h in range(HEADS)]

        def rel_phase(b, h):
            """q @ rel tables -> skewed+stacked [128, rc, {w,h}, 32] bf16 tiles."""
            ps = ps_big.tile([128, L], FP32, tag="big")
            for rc in range(RC):
                q_ch = qT[(b, h)][:, rc * 128:(rc + 1) * 128]
                nc.tensor.matmul(ps[:, rc * NREL:(rc + 1) * NREL], q_ch, relwT,
                                 start=True, stop=True)
                nc.tensor.matmul(ps[:, 512 + rc * NREL:512 + (rc + 1) * NREL],
                                 q_ch, relhT, start=True, stop=True)
            rel_sb = rel_p.tile([128, 1008], BF16, tag="rel_sb")
            nc.scalar.activation(rel_sb[:, 0:504], ps[:, 0:504], AF.Copy)
            nc.scalar.activation(rel_sb[:, 504:1008], ps[:, 512:1016], AF.Copy)
            return rel_sb

        def rel_fetch(rel_sb):
            """DRAM round-trip: skew-read rel_sb back as [128, rc, {w,h}, 32].

            Emitted AFTER the current pair's x-bar transposes: the rd write
            blocks the in-order SP sequencer until rel_sb is ready, so
            putting it first delayed the transposes (and PE's av) ~2.7us
            per pair.
            """
            if "noskew" in ABLATE:
                # timing-only ablation: skip the DRAM round-trip + gathers
                relwh = rel_p.tile([128, RC, 2, 32], BF16, tag="relwh")
                nc.vector.tensor_copy(
                    relwh.rearrange("p a b c -> p (a b c)"), rel_sb[:, 0:512])
                return relwh
            rd = dram_p.tile([128, 1008], BF16, tag="rel_dram")
            nc.scalar.dma_start(out=rd, in_=rel_sb)

            rd_ap = rd[:, :]
            base_t, base_off = rd_ap.tensor, rd_ap.offset
            assert [list(p) for p in rd_ap.ap] == [[1008, 128], [1, 1008]], rd_ap.ap

            # Read back long CONTIGUOUS per-partition runs with the diagonal
            # (per-partition skew offset) baked into the partition stride,
            # then compact the 63/59-strided 32-element windows on-chip with
            # strided DVE copies. Versus gathering the windows directly from
            # DRAM this turns 64B DMA lines into 1008B lines — the fine lines
            # collapse under 8-core HBM load (~0.26us/iter 1-core ->
            # ~17ms/iter 8-core).
            #   width:  R[p,c] = flat[p*1008 + c + 31 - (p%32)]
            #           -> rw_skew[p,rc,j] = R[p, rc*63 + j]
            #   height: R[p,c] = flat[p*1008 + 504 + c + 31 - (p//32)]
            #           -> rh_skew[p,rc,i] = R[p, rc*59 + i]
            wlong = rel_p.tile([128, 504], BF16, tag="wlong")
            hlong = rel_p.tile([128, 472], BF16, tag="hlong")
            # both gathers on SP: their issuance blocks the in-order HWDGE
            # sequencer until the rd WRITE completes (~2us); SP has nothing
            # due until the first x-bar transpose ~2.3us into the pair,
            # while on ACT the same block delayed Exp -> PSUM rotation.
            with nc.allow_non_contiguous_dma(reason="rel-pos skew diagonal read"):
                src_w = AP(base_t, base_off + 31,
                           [[32 * 1008, 4], [1008 - 1, 32], [1, 504]])
                nc.sync.dma_start(out=wlong[:, :], in_=src_w)
                src_h = AP(base_t, base_off + 504 + 31,
                           [[32 * 1008 - 1, 4], [1008, 32], [1, 445]])
                nc.sync.dma_start(out=hlong[:, 0:445], in_=src_h)
            relwh = rel_p.tile([128, RC, 2, 32], BF16, tag="relwh")
            nc.vector.tensor_copy(
                relwh[:, :, 0, :],
                wlong.rearrange("p (a b) -> p a b", a=RC)[:, :, 0:32])
            nc.vector.tensor_copy(
                relwh[:, :, 1, :],
                hlong.rearrange("p (a b) -> p a b", a=RC)[:, :, 0:32])
            return relwh

        def main_phase(b, h, relwh):
            # all 8 skewed rel tiles ([64,128] via PE transpose + DVE copy)
            # up front, so the qk/rel matmul stream below never stalls on a
            # mid-stream DVE copy
            relTs = []
            for rc in range(RC):
                psr = ps_wt.tile([64, 128], BF16, tag="ps_wt")
                nc.tensor.transpose(
                    psr, relwh[:, rc, :, :].rearrange("p a b -> p (a b)"), ident)
                relT = small.tile([64, 128], BF16, tag=f"relT{rc}")
                nc.vector.tensor_copy(relT, psr)
                relTs.append(relT)
            Wt = []
            for rc in range(RC):
                ps_l = ps_big.tile([128, L], FP32, tag="big")
                q_ch = qT[(b, h)][:, rc * 128:(rc + 1) * 128]
                w_sb = pair_p.tile([128, L], BF16, tag=f"W{rc}")
                for s in (slice(0, 512), slice(512, 1024)):
                    nc.tensor.matmul(ps_l[:, s], q_ch, kT[(b, h)][:, s],
                                     start=True, stop=False)
                    nc.tensor.matmul(ps_l[:, s], relTs[rc], sel[:, s],
                                     start=False, stop=True)
                den = small.tile([128, 1], FP32, tag=f"den{rc}")
                nc.scalar.activation(w_sb, ps_l, AF.Exp, accum_out=den)
                rden = small.tile([128, 1], FP32, tag=f"rden{rc}")
                nc.vector.reciprocal(rden, den)
                nc.vector.tensor_scalar_mul(w_sb, w_sb, rden)
                Wt.append(w_sb)
            return Wt

        def issue_wt_transposes(Wt, big_wt):
            # X-bar DMA transpose per row chunk: W[rc] [128x, 1024y] ->
            # big_wt[y_lo, cc, rc, x] so the av matmul's B operand for
            # y-chunk cc is the contiguous slice big_wt[:, cc*1024:+1024].
            # Offloads all 64 [128,128] block transposes per pair from the
            # PE (and their PSUM->SBUF copies from the DVE) onto the DMA
            # x-bar, which is otherwise idle.
            bw = big_wt.rearrange("p (a b c) -> p a b c", a=RC, b=RC)
            for rc in range(RC):
                nc.sync.dma_start_transpose(bw[:, :, rc, :], Wt[rc])

        def av_phase(b, h, big_wt):
            o_sb = out_p.tile([128, L], FP32, tag="o_sb")
            ps_o = ps_av.tile([128, L], FP32, tag="ps_av")
            # x-chunk-major: out[:, rc'] = sum_cc V_cc @ W^T[cc, x in rc'].
            # Chunk rc' only needs x-bar transpose rc', so the av stream
            # starts as soon as W[0] is transposed instead of waiting for
            # all 8 (which stalled PE ~2us at every pair boundary).
            for rcp in range(RC):
                for cc in range(RC):
                    v_ch = vT[(b, cc)][:, h * 128:(h + 1) * 128]
                    nc.tensor.matmul(
                        ps_o[:, rcp * 128:(rcp + 1) * 128],
                        v_ch,
                        big_wt[:, cc * L + rcp * 128:cc * L + (rcp + 1) * 128],
                        start=(cc == 0), stop=(cc == RC - 1))
            nc.vector.tensor_copy(o_sb, ps_o)
            return o_sb

        def issue_out_dma(b, h, o_sb):
            # out[b, h*128+d, l] <- o_sb[d, l]. Emitted one pair LATE (after
            # the next pair's x-bar transposes) so its fat SP issuance slice
            # doesn't delay them — that stalled PE ~2.7us per pair.
            nc.sync.dma_start(
                out=AP(out[b].tensor, out[b].offset + h * 128 * L,
                       [[L, 128], [1, L]]),
                in_=o_sb)

        # one-pair-deep rel lookahead: the DRAM round-trip + skew gathers for
        # pair i+1 are in flight while pair i computes
        load_fmap(1)
        project(0)
        relwh = rel_fetch(rel_phase(*pairs[0]))
        project(1)
        pending_out = None
        for i, (b, h) in enumerate(pairs):
            # next pair's rel matmuls FIRST: their PSUM buffer rotates in at
            # pair start (not after all 8 logits chunks), so the rel chain
            # completes mid-pair instead of stalling the next pair's sel
            # matmuls at the boundary. The DRAM round-trip (rel_fetch) is
            # emitted after this pair's transposes — see rel_fetch.
            relwh_next = (rel_fetch(rel_phase(*pairs[i + 1]))
                          if i + 1 < len(pairs) else None)
            Wt = main_phase(b, h, relwh)
            big_wt = wt_p.tile([128, RC * L], BF16, tag="bigwt")
            issue_wt_transposes(Wt, big_wt)
            o_sb = av_phase(b, h, big_wt)
            if pending_out is not None:
                issue_out_dma(*pending_out)
            pending_out = (b, h, o_sb)
            relwh = relwh_next
        issue_out_dma(*pending_out)


_NC_CACHE = None


def get_nc():
    global _NC_CACHE
    if _NC_CACHE is None:
        _NC_CACHE = build_bass()
    return _NC_CACHE


def kernel(featuremap, w_qk, w_v, rel_height, rel_width):
    B, C_, H_, W_ = featuremap.shape
    nc = get_nc()
    fm = np.ascontiguousarray(featuremap, np.float32).reshape(B, C_, H_ * W_)
    common = {
        "w_qk": np.ascontiguousarray(w_qk, np.float32),
        "w_v": np.ascontiguousarray(w_v, np.float32),
        "rel_height": np.ascontiguousarray(rel_height, np.float32),
        "rel_width": np.ascontiguousarray(rel_width, np.float32),
    }
    in_maps = [
        {"fmap": fm[i * B_PER_CORE:(i + 1) * B_PER_CORE], **common}
        for i in range(NCORES)
    ]
    res = run_bass_kernel_spmd(nc, in_maps, list(range(NCORES))).results
    outs = [res[i]["out"].reshape(B_PER_CORE, HEADS * D, H_, W_) for i in range(NCORES)]
    return np.concatenate(outs, axis=0).astype(np.float32)



# revision 57
# speedup vs baseline: 8.6899x; 8.6899x over previous
"""BotNet-style multi-head 2D attention with relative position logits, on 8 trn2 cores.

Distribution: data-parallel over batch (B=16 -> 2 per core); all 4 heads +
the rel-pos skew handled on-core.

Per (batch, head) pair the kernel computes, fully on-chip:
    logits = (q*SCALE) @ k^T + skew_w(q @ relw^T) + skew_h(q @ relh^T)
    W      = exp(logits);  W /= rowsum(W)   (softmax without max-subtract:
             logits are O(10) here, exp() is safe in fp32)
    out^T  = V^T @ W^T     (accumulated over key chunks in PSUM)

The rel-pos skew (per-query-row shift) is done with a DRAM round-trip whose
read-back access pattern bakes in the shift, then the per-row [64,128] skewed
tile is added into the logits PSUM via a matmul against a constant 0/1
selector matrix (contraction over the 32 width / 32 height rel positions).
"""

import numpy as np
import ml_dtypes

import concourse.bass as bass
import concourse.mybir as mybir
import concourse.tile as tile
from concourse import bacc
from concourse.ap import AP
from concourse.bass_utils import run_bass_kernel_spmd

FP32 = mybir.dt.float32
BF16 = mybir.dt.bfloat16
AF = mybir.ActivationFunctionType

import os
ABLATE = set(os.environ.get("KERNEL_ABLATE", "").split(","))

NCORES = 8
B_PER_CORE = 2
HEADS = 4
D = 128          # qk and v head dim
C = 512          # input channels
H = W = 32
L = H * W        # 1024 tokens
RC = L // 128    # 8 row chunks of 128 tokens
CC = C // 128    # 4 contraction chunks for the projections
SCALE = D ** (-0.5)
NREL = 2 * W - 1  # 63


def _sel_matrix():
    # sel[k, i*32+j]: k<32 -> (j == k); k>=32 -> (i == k-32)
    sel = np.zeros((64, L), np.float32)
    ii, jj = np.divmod(np.arange(L), W)
    for k in range(32):
        sel[k, jj == k] = 1.0
        sel[32 + k, ii == k] = 1.0
    return sel.astype(ml_dtypes.bfloat16)


def build_bass(iters=1):
    nc = bacc.Bacc()
    fmap = nc.declare_dram_parameter("fmap", [B_PER_CORE, C, L], FP32, isOutput=False)
    wqk = nc.declare_dram_parameter("w_qk", [2 * HEADS * D, C], FP32, isOutput=False)
    wv = nc.declare_dram_parameter("w_v", [HEADS * D, C], FP32, isOutput=False)
    relh = nc.declare_dram_parameter("rel_height", [NREL, D], FP32, isOutput=False)
    relw = nc.declare_dram_parameter("rel_width", [NREL, D], FP32, isOutput=False)
    out = nc.declare_dram_parameter("out", [B_PER_CORE, HEADS * D, L], FP32, isOutput=True)

    sel_const = nc.inline_tensor(_sel_matrix(), name="sel_const")
    ident_const = nc.inline_tensor(np.eye(128, dtype=ml_dtypes.bfloat16), name="ident_const")

    with tile.TileContext(nc) as tc:
        if iters == 1:
            _body(tc, fmap, wqk, wv, relh, relw, out, sel_const, ident_const)
        else:
            with tc.For_i(0, iters, 1):
                _body(tc, fmap, wqk, wv, relh, relw, out, sel_const, ident_const)
    nc.finalize()
    return nc


def _body(tc, fmap, wqk, wv, relh, relw, out, sel_const, ident_const):
    nc = tc.nc
    import contextlib

    ctx = contextlib.ExitStack()
    with ctx:
        persist = ctx.enter_context(tc.tile_pool(name="persist", bufs=1))
        batch_p = ctx.enter_context(tc.tile_pool(name="batch", bufs=2))
        pair_p = ctx.enter_context(tc.tile_pool(name="pair", bufs=2))
        rel_p = ctx.enter_context(tc.tile_pool(name="rel", bufs=3))
        out_p = ctx.enter_context(tc.tile_pool(name="out", bufs=2))
        wt_p = ctx.enter_context(tc.tile_pool(name="wtsb", bufs=2))
        small = ctx.enter_context(tc.tile_pool(name="small", bufs=2))
        dram_p = ctx.enter_context(tc.tile_pool(name="dram", bufs=3, space="DRAM"))

        ps_big = ctx.enter_context(tc.tile_pool(name="ps_big", bufs=2, space="PSUM"))
        ps_wt = ctx.enter_context(tc.tile_pool(name="ps_wt", bufs=2, space="PSUM"))
        ps_av = ctx.enter_context(tc.tile_pool(name="ps_av", bufs=1, space="PSUM"))

        # ---- constants to SBUF ----
        ident = persist.tile([128, 128], BF16, tag="ident")
        nc.sync.dma_start(out=ident, in_=ident_const[:])
        sel = persist.tile([64, L], BF16, tag="sel")
        nc.sync.dma_start(out=sel, in_=sel_const[:])

        # ---- weight prep: transpose + cast to bf16 (scale folded into q) ----
        # wqk rows: [0,512) = q (scaled), [512,1024) = k
        # single gpsimd DMA per weight (casts fp32->bf16 in flight):
        # [128, oc*512+c] <- w[oc*128+p, c]
        hwload = "hwload" in ABLATE
        wq_all = persist.tile([128, 8 * C], BF16, tag="wqldb")
        wv_all = persist.tile([128, 4 * C], BF16, tag="wvldb")
        if hwload:
            # HWDGE fp32 loads + engine casts: keeps the (slow) SWDGE
            # descriptor generation off the kernel-start critical path
            stage_p = ctx.enter_context(tc.tile_pool(name="stage", bufs=1))
            wq32 = stage_p.tile([128, 8 * C], FP32, tag="wstage")
            nc.sync.dma_start(
                out=wq32.rearrange("p (a c) -> p a c", a=8),
                in_=wqk[:].rearrange("(a p) c -> p a c", p=128))
            nc.vector.tensor_copy(wq_all, wq32)
            wv32 = stage_p.tile([128, 8 * C], FP32, tag="wstage")
            nc.sync.dma_start(
                out=wv32[:, 0:4 * C].rearrange("p (a c) -> p a c", a=4),
                in_=wv[:].rearrange("(a p) c -> p a c", p=128))
            nc.vector.tensor_copy(wv_all, wv32[:, 0:4 * C])
        else:
            nc.gpsimd.dma_start(
                out=wq_all.rearrange("p (a c) -> p a c", a=8),
                in_=wqk[:].rearrange("(a p) c -> p a c", p=128))
        wq_bf = [wq_all[:, oc * C:(oc + 1) * C] for oc in range(8)]
        wv_bf = [wv_all[:, oc * C:(oc + 1) * C] for oc in range(4)]

        # fmap loads issued on the Pool (SWDGE) queue right after wq so the
        # batch-0 chunks land while PE transposes weights; wv (not needed
        # until the v projections ~30us in) queues behind them
        fm_tiles = {}

        def load_fmap(b):
            fm_bf = []
            for cc in range(CC):
                fbf = batch_p.tile([128, L], BF16, tag=f"fmbf_{cc}")
                if hwload:
                    f32 = batch_p.tile([128, L], FP32, tag="fm32")
                    nc.sync.dma_start(out=f32,
                                      in_=fmap[b, cc * 128:(cc + 1) * 128, :])
                    nc.vector.tensor_copy(fbf, f32)
                else:
                    nc.gpsimd.dma_start(out=fbf,
                                        in_=fmap[b, cc * 128:(cc + 1) * 128, :])
                fm_bf.append(fbf)
            fm_tiles[b] = fm_bf

        load_fmap(0)
        if not hwload:
            nc.gpsimd.dma_start(
                out=wv_all.rearrange("p (a c) -> p a c", a=4),
                in_=wv[:].rearrange("(a p) c -> p a c", p=128))

        wqkT = []   # per cc: [128(c), 1024(o)] bf16, q-half pre-scaled
        for cc in range(CC):
            ps = ps_wt.tile([128, 1024], BF16, tag="ps_wt")
            for oc in range(8):
                nc.tensor.transpose(
                    ps[:, oc * 128:(oc + 1) * 128],
                    wq_bf[oc][:, cc * 128:(cc + 1) * 128],
                    ident,
                )
            t = persist.tile([128, 1024], BF16, tag=f"wqkT{cc}")
            nc.vector.tensor_scalar_mul(t[:, 0:512], ps[:, 0:512], float(SCALE))
            nc.vector.tensor_copy(t[:, 512:1024], ps[:, 512:1024])
            wqkT.append(t)

        wvT = []    # per cc: [128(c), 512(o)] bf16
        for cc in range(CC):
            ps = ps_wt.tile([128, 1024], BF16, tag="ps_wt")
            for oc in range(4):
                nc.tensor.transpose(
                    ps[:, oc * 128:(oc + 1) * 128],
                    wv_bf[oc][:, cc * 128:(cc + 1) * 128],
                    ident,
                )
            t = persist.tile([128, 512], BF16, tag=f"wvT{cc}")
            nc.vector.tensor_copy(t, ps[:, 0:512])
            wvT.append(t)

        # rel tables transposed: [128(d), 63] bf16
        relT_tabs = []
        for name, src in (("relw", relw), ("relh", relh)):
            tbf = small.tile([NREL, D], BF16, tag=f"{name}b")
            nc.gpsimd.dma_start(out=tbf, in_=src[:])
            ps = ps_wt.tile([128, 1024], BF16, tag="ps_wt")
            nc.tensor.transpose(ps[:, 0:NREL], tbf, ident[0:NREL, 0:NREL])
            t = persist.tile([128, NREL], BF16, tag=f"{name}T")
            nc.scalar.activation(t, ps[:, 0:NREL], AF.Copy)
            relT_tabs.append(t)
        relwT, relhT = relT_tabs

        # ---- projections (emitted per batch; pair-0's rel round-trip is
        # issued between the two batches so its DRAM latency hides under
        # batch-1's projection matmuls) ----
        qT = {}   # (b, h) -> [128(d), 1024(l)] bf16  (pre-scaled by SCALE)
        kT = {}
        vT = {}   # (b, lc) -> [128(l), 512(h*d)] bf16

        def project(b):
            fm_bf = fm_tiles[b]
            # q/k: out[o_chunk, l] ; o = (q: h*128+d | k: 512 + h*128+d)
            for oc in range(8):
                ps = ps_big.tile([128, L], FP32, tag="big")
                for s in (slice(0, 512), slice(512, 1024)):
                    for cc in range(CC):
                        nc.tensor.matmul(
                            ps[:, s],
                            wqkT[cc][:, oc * 128:(oc + 1) * 128],
                            fm_bf[cc][:, s],
                            start=(cc == 0),
                            stop=(cc == CC - 1),
                        )
                dst = batch_p.tile([128, L], BF16,
                                   tag=f"{'q' if oc < 4 else 'k'}T{oc % 4}")
                if oc < 4:
                    nc.vector.tensor_copy(dst, ps)
                    qT[(b, oc)] = dst
                else:
                    nc.scalar.activation(dst, ps, AF.Copy)
                    kT[(b, oc - 4)] = dst
            # v^T: out[l_chunk, h*d]
            for lc in range(RC):
                ps = ps_big.tile([128, L], FP32, tag="big")
                for cc in range(CC):
                    nc.tensor.matmul(
                        ps[:, 0:512],
                        fm_bf[cc][:, lc * 128:(lc + 1) * 128],
                        wvT[cc],
                        start=(cc == 0),
                        stop=(cc == CC - 1),
                    )
                dst = batch_p.tile([128, 512], BF16, tag=f"vT{lc}")
                nc.vector.tensor_copy(dst, ps[:, 0:512])
                vT[(b, lc)] = dst

        # ---- attention pairs ----
        pairs = [(b, h) for b in range(B_PER_CORE) for h in range(HEADS)]

        def rel_phase(b, h):
            """q @ rel tables -> skewed+stacked [128, rc, {w,h}, 32] bf16 tiles."""
            ps = ps_big.tile([128, L], FP32, tag="big")
            for rc in range(RC):
                q_ch = qT[(b, h)][:, rc * 128:(rc + 1) * 128]
                nc.tensor.matmul(ps[:, rc * NREL:(rc + 1) * NREL], q_ch, relwT,
                                 start=True, stop=True)
                nc.tensor.matmul(ps[:, 512 + rc * NREL:512 + (rc + 1) * NREL],
                                 q_ch, relhT, start=True, stop=True)
            rel_sb = rel_p.tile([128, 1008], BF16, tag="rel_sb")
            nc.scalar.activation(rel_sb[:, 0:504], ps[:, 0:504], AF.Copy)
            nc.scalar.activation(rel_sb[:, 504:1008], ps[:, 512:1016], AF.Copy)
            return rel_sb

        def rel_fetch(rel_sb):
            """DRAM round-trip: skew-read rel_sb back as [128, rc, {w,h}, 32].

            Emitted AFTER the current pair's x-bar transposes: the rd write
            blocks the in-order SP sequencer until rel_sb is ready, so
            putting it first delayed the transposes (and PE's av) ~2.7us
            per pair.
            """
            if "noskew" in ABLATE:
                # timing-only ablation: skip the DRAM round-trip + gathers
                relwh = rel_p.tile([128, RC, 2, 32], BF16, tag="relwh")
                nc.vector.tensor_copy(
                    relwh.rearrange("p a b c -> p (a b c)"), rel_sb[:, 0:512])
                return relwh
            rd = dram_p.tile([128, 1008], BF16, tag="rel_dram")
            nc.scalar.dma_start(out=rd, in_=rel_sb)

            rd_ap = rd[:, :]
            base_t, base_off = rd_ap.tensor, rd_ap.offset
            assert [list(p) for p in rd_ap.ap] == [[1008, 128], [1, 1008]], rd_ap.ap

            # Read back long CONTIGUOUS per-partition runs with the diagonal
            # (per-partition skew offset) baked into the partition stride,
            # then compact the 63/59-strided 32-element windows on-chip with
            # strided DVE copies. Versus gathering the windows directly from
            # DRAM this turns 64B DMA lines into 1008B lines — the fine lines
            # collapse under 8-core HBM load (~0.26us/iter 1-core ->
            # ~17ms/iter 8-core).
            #   width:  R[p,c] = flat[p*1008 + c + 31 - (p%32)]
            #           -> rw_skew[p,rc,j] = R[p, rc*63 + j]
            #   height: R[p,c] = flat[p*1008 + 504 + c + 31 - (p//32)]
            #           -> rh_skew[p,rc,i] = R[p, rc*59 + i]
            wlong = rel_p.tile([128, 504], BF16, tag="wlong")
            hlong = rel_p.tile([128, 472], BF16, tag="hlong")
            # both gathers on SP: their issuance blocks the in-order HWDGE
            # sequencer until the rd WRITE completes (~2us); SP has nothing
            # due until the first x-bar transpose ~2.3us into the pair,
            # while on ACT the same block delayed Exp -> PSUM rotation.
            with nc.allow_non_contiguous_dma(reason="rel-pos skew diagonal read"):
                src_w = AP(base_t, base_off + 31,
                           [[32 * 1008, 4], [1008 - 1, 32], [1, 504]])
                nc.sync.dma_start(out=wlong[:, :], in_=src_w)
                src_h = AP(base_t, base_off + 504 + 31,
                           [[32 * 1008 - 1, 4], [1008, 32], [1, 445]])
                nc.sync.dma_start(out=hlong[:, 0:445], in_=src_h)
            relwh = rel_p.tile([128, RC, 2, 32], BF16, tag="relwh")
            nc.vector.tensor_copy(
                relwh[:, :, 0, :],
                wlong.rearrange("p (a b) -> p a b", a=RC)[:, :, 0:32])
            nc.vector.tensor_copy(
                relwh[:, :, 1, :],
                hlong.rearrange("p (a b) -> p a b", a=RC)[:, :, 0:32])
            return relwh

        def main_phase(b, h, relwh):
            # all 8 skewed rel tiles ([64,128] via PE transpose + DVE copy)
            # up front, so the qk/rel matmul stream below never stalls on a
            # mid-stream DVE copy
            relTs = []
            for rc in range(RC):
                psr = ps_wt.tile([64, 128], BF16, tag="ps_wt")
                nc.tensor.transpose(
                    psr, relwh[:, rc, :, :].rearrange("p a b -> p (a b)"), ident)
                relT = small.tile([64, 128], BF16, tag=f"relT{rc}")
                nc.vector.tensor_copy(relT, psr)
                relTs.append(relT)
            Wt = []
            for rc in range(RC):
                ps_l = ps_big.tile([128, L], FP32, tag="big")
                q_ch = qT[(b, h)][:, rc * 128:(rc + 1) * 128]
                w_sb = pair_p.tile([128, L], BF16, tag=f"W{rc}")
                for s in (slice(0, 512), slice(512, 1024)):
                    nc.tensor.matmul(ps_l[:, s], q_ch, kT[(b, h)][:, s],
                                     start=True, stop=False)
                    nc.tensor.matmul(ps_l[:, s], relTs[rc], sel[:, s],
                                     start=False, stop=True)
                den = small.tile([128, 1], FP32, tag=f"den{rc}")
                nc.scalar.activation(w_sb, ps_l, AF.Exp, accum_out=den)
                rden = small.tile([128, 1], FP32, tag=f"rden{rc}")
                nc.vector.reciprocal(rden, den)
                nc.vector.tensor_scalar_mul(w_sb, w_sb, rden)
                Wt.append(w_sb)
            return Wt

        def issue_wt_transposes(Wt, big_wt):
            # X-bar DMA transpose per row chunk: W[rc] [128x, 1024y] ->
            # big_wt[y_lo, cc, rc, x] so the av matmul's B operand for
            # y-chunk cc is the contiguous slice big_wt[:, cc*1024:+1024].
            # Offloads all 64 [128,128] block transposes per pair from the
            # PE (and their PSUM->SBUF copies from the DVE) onto the DMA
            # x-bar, which is otherwise idle.
            bw = big_wt.rearrange("p (a b c) -> p a b c", a=RC, b=RC)
            if "xbartrans" in ABLATE:
                # X-bar variant: models ~14us faster in CoreSim but is
                # ~100us SLOWER on HW at 1 core (real per-DMA fixed costs
                # on the critical path); kept for reference
                for rc in range(RC):
                    nc.sync.dma_start_transpose(bw[:, :, rc, :], Wt[rc])
                return
            for rc in range(RC):
                psw = ps_wt.tile([128, 1024], BF16, tag="ps_wt")
                for cc in range(RC):
                    nc.tensor.transpose(
                        psw[:, cc * 128:(cc + 1) * 128],
                        Wt[rc][:, cc * 128:(cc + 1) * 128], ident)
                nc.vector.tensor_copy(
                    bw[:, :, rc, :],
                    psw.rearrange("p (a c) -> p a c", a=RC))

        def av_phase(b, h, big_wt):
            o_sb = out_p.tile([128, L], FP32, tag="o_sb")
            ps_o = ps_av.tile([128, L], FP32, tag="ps_av")
            # x-chunk-major: out[:, rc'] = sum_cc V_cc @ W^T[cc, x in rc'].
            # Chunk rc' only needs x-bar transpose rc', so the av stream
            # starts as soon as W[0] is transposed instead of waiting for
            # all 8 (which stalled PE ~2us at every pair boundary).
            for rcp in range(RC):
                for cc in range(RC):
                    v_ch = vT[(b, cc)][:, h * 128:(h + 1) * 128]
                    nc.tensor.matmul(
                        ps_o[:, rcp * 128:(rcp + 1) * 128],
                        v_ch,
                        big_wt[:, cc * L + rcp * 128:cc * L + (rcp + 1) * 128],
                        start=(cc == 0), stop=(cc == RC - 1))
            nc.vector.tensor_copy(o_sb, ps_o)
            return o_sb

        def issue_out_dma(b, h, o_sb):
            # out[b, h*128+d, l] <- o_sb[d, l]. Emitted one pair LATE (after
            # the next pair's x-bar transposes) so its fat SP issuance slice
            # doesn't delay them — that stalled PE ~2.7us per pair.
            nc.sync.dma_start(
                out=AP(out[b].tensor, out[b].offset + h * 128 * L,
                       [[L, 128], [1, L]]),
                in_=o_sb)

        # one-pair-deep rel lookahead: the DRAM round-trip + skew gathers for
        # pair i+1 are in flight while pair i computes
        load_fmap(1)
        project(0)
        relwh = rel_fetch(rel_phase(*pairs[0]))
        project(1)
        pending_out = None
        for i, (b, h) in enumerate(pairs):
            # next pair's rel matmuls FIRST: their PSUM buffer rotates in at
            # pair start (not after all 8 logits chunks), so the rel chain
            # completes mid-pair instead of stalling the next pair's sel
            # matmuls at the boundary. The DRAM round-trip (rel_fetch) is
            # emitted after this pair's transposes — see rel_fetch.
            relwh_next = (rel_fetch(rel_phase(*pairs[i + 1]))
                          if i + 1 < len(pairs) else None)
            Wt = main_phase(b, h, relwh)
            big_wt = wt_p.tile([128, RC * L], BF16, tag="bigwt")
            issue_wt_transposes(Wt, big_wt)
            o_sb = av_phase(b, h, big_wt)
            if pending_out is not None:
                issue_out_dma(*pending_out)
            pending_out = (b, h, o_sb)
            relwh = relwh_next
        issue_out_dma(*pending_out)


_NC_CACHE = None


def get_nc():
    global _NC_CACHE
    if _NC_CACHE is None:
        _NC_CACHE = build_bass()
    return _NC_CACHE


def kernel(featuremap, w_qk, w_v, rel_height, rel_width):
    B, C_, H_, W_ = featuremap.shape
    nc = get_nc()
    fm = np.ascontiguousarray(featuremap, np.float32).reshape(B, C_, H_ * W_)
    common = {
        "w_qk": np.ascontiguousarray(w_qk, np.float32),
        "w_v": np.ascontiguousarray(w_v, np.float32),
        "rel_height": np.ascontiguousarray(rel_height, np.float32),
        "rel_width": np.ascontiguousarray(rel_width, np.float32),
    }
    in_maps = [
        {"fmap": fm[i * B_PER_CORE:(i + 1) * B_PER_CORE], **common}
        for i in range(NCORES)
    ]
    res = run_bass_kernel_spmd(nc, in_maps, list(range(NCORES))).results
    outs = [res[i]["out"].reshape(B_PER_CORE, HEADS * D, H_, W_) for i in range(NCORES)]
    return np.concatenate(outs, axis=0).astype(np.float32)

